# revision 8
# baseline (speedup 1.0000x reference)
"""MQA causal attention block (b=2, n=2048, d=1024, h=16, dh=64) on 8
Trainium2 NeuronCores.

Sharding: data-parallel over batch (2) x tensor-parallel over head groups
(4 heads/core). Each core computes, for its batch b and heads [4g, 4g+4):
  qT = Wq_g @ x^T in fp8e4m3 DoubleRow  [256, 2048]  (UNscaled; the 1/32
    softmax scale is folded into the exp stage; ISA caps DR moving
    patterns at 512 elements -> 256-col outputs, 8 matmuls per block)
  kT|vT = [Wk|Wv]^T proj fp16           [128, 2048]   (k rows 0:64, v 64:128)
  attention, chunk-major per head pair (t2), software-pipelined:
    S^T(c) for both heads via row-tiled fp16 matmuls (even head on PE
    rows 0:63, odd head on rows 64:127 -> concurrent);
    p = exp(S*SCALE): split across engines - Scalar ACT (exact, scale arg)
    on even chunk idx + pair-final, custom-DVE cubic Taylor
    ((e*S^3/6 + S^2/2)*e + S)*e + 1 on odd idx (err < 0.4% at |e*S|<=0.55);
    PV(c-1) merged across both heads (single 1024-free matmul) emitted
    after S(c) so the PE never stalls on the exp latency.
    vo carries [v | 1] so softmax denominators accumulate in oa row 64.
  normalize: rinv = 1/D (reciprocal + K=1 f32r broadcast matmul),
    ot = oa * rinv  (fp16)
  y^T block = ot @ WfcT (fp16); fc units interleave into the attention
  stream as PE fillers; ysb copies alternate ACT/DVE.
Host sums the 4 partial y per batch and adds bfc.
"""
import os
import sys

for _p in ("/opt/trn_rl_repo",):
    if _p not in sys.path:
        sys.path.insert(0, _p)

import numpy as np

import concourse.bass as bass  # noqa: F401
import concourse.mybir as mybir
import concourse.tile as tile
from concourse import bacc
from concourse.bass_utils import run_bass_kernel_spmd

# bass_utils unconditionally imports antenv.axon_hooks when tracing under
# axon; provide a no-op registry if the image doesn't ship one so a traced
# run degrades to "no profile" instead of crashing.
try:
    import antenv.axon_hooks  # noqa: F401
except Exception:  # pragma: no cover
    import types

    _m = types.ModuleType("antenv.axon_hooks")
    _m._hook = None
    _m.set_axon_ntff_profile_hook = lambda h: setattr(_m, "_hook", h)
    _m.get_axon_ntff_profile_hook = lambda: getattr(_m, "_hook", None)
    sys.modules["antenv.axon_hooks"] = _m

F32 = mybir.dt.float32
F32R = mybir.dt.float32r
F16 = mybir.dt.float16
F8 = mybir.dt.float8e4
F8NP = mybir.dt.np(F8)
EXP = mybir.ActivationFunctionType.Exp
DR = mybir.MatmulPerfMode.DoubleRow

NH, DH, D, N, NB = 16, 64, 1024, 2048, 2
HPC = 4          # heads per core (2 pairs)
SCALE = D ** (-0.5)
NIC = N // 512   # 4 query blocks of 512
NDC = D // 128   # 8 contraction chunks

_compiled = None
_last_results = None
_warmed = False
last_exec_time_ns = None

_EXP_CUBIC = None


def _register_exp_cubic():
    """Runtime-register a custom DVE op computing the cubic Taylor of
    exp(SCALE*x): out = ((x*c3 + c2)*x + c1)*x + 1. Appended to dve_ops.OPS
    (never reordering existing rows) with the sha pinned from this
    process's own lower() output."""
    global _EXP_CUBIC
    if _EXP_CUBIC is not None:
        return _EXP_CUBIC
    from concourse import dve_ops as _dvo
    from concourse.dve_spec import C0, C1, C2, One, Spec, Src0, lower
    from concourse.dve_uop import DveOpSpec

    name = "EXP_CUBIC_ANT"
    if name in _dvo._SUB_OPCODE_FOR_NAME:
        _EXP_CUBIC = next(o for o in _dvo.OPS if o.name == name)
        return _EXP_CUBIC
    body = ((Src0 * C0 + C1) * Src0 + C2) * Src0 + One
    spec = Spec(
        body=body,
        reference=lambda in0, in1, s0, s1, imm2: (
            ((in0 * s0 + s1) * in0 + imm2) * in0 + 1.0
        ),
    )
    shas = {}
    for ver in ("v3", "v4"):
        uops = lower(spec, ver=ver)
        shas[ver] = DveOpSpec(name=name, opcode=0, uops=uops, rd1_en=False).sha(ver)
    op = _dvo.DveOp(name, spec, subdim=False, uops_sha=shas)
    _dvo.OPS.append(op)
    _dvo._SUB_OPCODE_FOR_NAME[name] = _dvo._CUSTOM_DVE_ROW_BASE + len(_dvo.OPS) - 1
    _dvo.CUSTOM_DVE_SPECS[name] = spec
    _EXP_CUBIC = op
    return op


def _build():
    if os.environ.get("KERNEL_LDW_OPT"):
        import concourse.bass_utils as _bu
        if not getattr(_bu, "_ldw_patched", False):
            _orig = _bu.run_command
            def _patched(argv, **kw):
                argv = ["--enable-ldw-opt=true" if a == "--enable-ldw-opt=false" else a
                        for a in argv]
                return _orig(argv, **kw)
            _bu.run_command = _patched
            _bu._ldw_patched = True
    use_dve_exp = not os.environ.get("KERNEL_NO_DVE_EXP")
    # ISA caps the matmul moving pattern at 512 elements, so a both-heads
    # 1024-free PV is not encodable; keep per-head PV matmuls
    use_pv_merge = bool(os.environ.get("KERNEL_PV_MERGE"))
    use_q_fp8 = not os.environ.get("KERNEL_NO_Q_FP8")
    exp_op = _register_exp_cubic() if use_dve_exp else None
    C3C = float(SCALE ** 3 / 6.0)
    C2C = float(SCALE ** 2 / 2.0)

    nc = bacc.Bacc("TRN2", target_bir_lowering=False, debug=False, num_devices=8)
    xT_d = nc.dram_tensor("xT", [D, N], F16, kind="ExternalInput").ap()
    xT8_d = nc.dram_tensor("xT8", [D, N], F8, kind="ExternalInput").ap()
    wq8_d = nc.dram_tensor("wq8", [D, HPC * DH], F8, kind="ExternalInput").ap()
    wkv_d = nc.dram_tensor("wkv", [D, 2 * DH], F16, kind="ExternalInput").ap()
    wfc_d = nc.dram_tensor("wfc", [HPC * DH, D], F16, kind="ExternalInput").ap()
    y_d = nc.dram_tensor("y", [N, D], F16, kind="ExternalOutput").ap()

    with tile.TileContext(nc) as tc:
        with nc.allow_low_precision(reason="fp8/float32r bits"), tc.tile_pool(
            name="sb", bufs=1
        ) as sb, tc.tile_pool(name="work", bufs=8) as wk, tc.tile_pool(
            name="out", bufs=4
        ) as ob, tc.tile_pool(name="ps", bufs=1, space="PSUM") as ps:
            # ---- persistent SBUF ----
            xt = sb.tile([128, NDC, N], F16, tag="xt")
            xt8 = sb.tile([128, NDC, N], F8, tag="xt8")
            wqt8 = sb.tile([128, NDC, HPC * DH], F8, tag="wqt8")
            wkvt = sb.tile([128, NDC, 2 * DH], F16, tag="wkvt")
            wfct = sb.tile([128, 2, D], F16, tag="wfct")
            kvt = sb.tile([128, N], F16, tag="kvt")   # rows 0:64 kT, 64:128 vT
            k2 = sb.tile([128, N], F16, tag="k2")     # rows 64:128 = kT copy
            vo = sb.tile([128, NDC * 2, DH + 2], F16, tag="vo")  # [v | 1] per chunk
            qt = sb.tile([128, 2, N], F16, tag="qt")  # head pairs on partitions
            ot = sb.tile([128, 2, N], F16, tag="ot")  # attn out^T, same layout
            ident = sb.tile([128, 128], F16, tag="ident")
            ones_row = sb.tile([1, DH], F32R, tag="ones_row")

            # ---- input DMA: fp8 q-side operands + wkv first (q proj and the
            # DMA-paced kv proj start early); fp16 x after; wfc last ----
            for di in range(NDC):
                nc.sync.dma_start(out=wkvt[:, di, :], in_=wkv_d[di * 128 : di * 128 + 128, :])
                nc.sync.dma_start(out=wqt8[:, di, :], in_=wq8_d[di * 128 : di * 128 + 128, :])
                for hf in range(2):
                    nc.sync.dma_start(
                        out=xt8[:, di, hf * N // 2 : (hf + 1) * N // 2],
                        in_=xT8_d[di * 128 : di * 128 + 128, hf * N // 2 : (hf + 1) * N // 2],
                    )
            for di in range(NDC):
                for hf in range(2):
                    nc.sync.dma_start(
                        out=xt[:, di, hf * N // 2 : (hf + 1) * N // 2],
                        in_=xT_d[di * 128 : di * 128 + 128, hf * N // 2 : (hf + 1) * N // 2],
                    )
            for t2 in range(2):
                nc.sync.dma_start(out=wfct[:, t2, :], in_=wfc_d[t2 * 128 : t2 * 128 + 128, :])
            from concourse.masks import make_identity
            make_identity(nc, ident[:, :])
            nc.vector.memset(ones_row[:, :].bitcast(F32), 1.0)

            # ---- PE warm-up + early ACT/DVE table load during the DMA wait ----
            wsc = sb.tile([128, 512], F16, tag="wsc")
            nc.vector.memset(wsc[:, :], 0.5)
            wact = wk.tile([1, 16], F16, tag="wact")
            nc.scalar.activation(wact[:, :], wsc[0:1, 0:16], EXP, scale=float(SCALE))
            if use_dve_exp:
                wdve = wk.tile([1, 16], F16, tag="wdve")
                nc.vector._custom_dve(
                    exp_op, out=wdve[:, :], in0=wsc[0:1, 16:32],
                    s0=C3C, s1=C2C, imm2=float(SCALE),
                )
            for wi in range(8):
                wps = ps.tile([128, 512], F32, tag="mmps", bufs=2)
                nc.tensor.matmul(wps[:, :], wsc[:, 0:128], wsc[:, :],
                                 start=True, stop=True)

            # ---- q projection, all 4 blocks up-front.  fp8 DoubleRow: 4
            # d-pair accumulations x 2 column halves (ISA caps the DR moving
            # pattern at 512 elements -> 256-col outputs).  Result cast to
            # fp16 qt; copies alternate ACT/DVE ----
            def _qproj(t2, icb, ci):
                qp = ps.tile([128, 512], F32, tag="mmps", bufs=2)
                if use_q_fp8:
                    # PSUM zeroing on start=True is bank-granular (2KB);
                    # both 256-col halves live in one bank, so only the very
                    # first matmul may carry start — the rest accumulate onto
                    # the pending-zeroed region
                    for dp in range(4):
                        for half in range(2):
                            c0 = icb * 512 + half * 256
                            nc.tensor.matmul(
                                qp[:, half * 256 : half * 256 + 256],
                                wqt8[:, 2 * dp : 2 * dp + 2, t2 * 128 : t2 * 128 + 128],
                                xt8[:, 2 * dp : 2 * dp + 2, c0 : c0 + 256],
                                start=(dp == 0 and half == 0),
                                stop=(dp == 3),
                                perf_mode=DR,
                                skip_group_check=True,
                            )
                else:
                    for di in range(NDC):
                        nc.tensor.matmul(
                            qp[:, :],
                            wqt8[:, di, t2 * 128 : t2 * 128 + 128],
                            xt8[:, di, icb * 512 : icb * 512 + 512],
                            start=(di == 0),
                            stop=(di == NDC - 1),
                            skip_group_check=True,
                        )
                dst = qt[:, t2, icb * 512 : icb * 512 + 512]
                if ci % 2 == 0:
                    nc.scalar.copy(dst, qp[:, :])
                else:
                    nc.vector.tensor_copy(dst, qp[:, :])

            qprojs = [(t2, icb) for icb in range(NIC) for t2 in range(2)]

            kvpa = ps.tile([128, 2, 512], F32, tag="stp", bufs=2)
            kvpb = ps.tile([128, 2, 512], F32, tag="stp", bufs=2)

            def _kv_di(di):
                for j4 in range(4):
                    acc = kvpa if j4 < 2 else kvpb
                    nc.tensor.matmul(
                        acc[:, j4 % 2, :],
                        wkvt[:, di, :],
                        xt[:, di, j4 * 512 : j4 * 512 + 512],
                        start=(di == 0),
                        stop=(di == NDC - 1),
                        skip_group_check=True,
                    )

            # first two q blocks lead (ready as soon as xt8 lands, ahead of
            # the first fp16 x chunk); then alternate with the kv chunks
            _qproj(*qprojs[0], 0)
            _qproj(*qprojs[1], 1)
            for di in range(NDC):
                _kv_di(di)
                if di + 2 < len(qprojs):
                    _qproj(*qprojs[di + 2], di + 2)

            # ---- kv evacuation + odd-head kT duplicate at base partition 64 ----
            for j4 in range(4):
                acc = kvpa if j4 < 2 else kvpb
                # scalar engine: idle at this point, and faster from PSUM
                nc.scalar.copy(kvt[:, j4 * 512 : j4 * 512 + 512], acc[:, j4 % 2, :])
            for j4 in range(4):
                nc.vector.tensor_copy(
                    k2[64:128, j4 * 512 : j4 * 512 + 512],
                    kvt[0:64, j4 * 512 : j4 * 512 + 512],
                )

            # ---- fc for one 128-row block x one 512-col half ----
            def _fc_unit(ib, fcn, tail=False):
                yp = ps.tile([128, 512], F32, tag="mmps", bufs=2)
                for t2 in range(2):
                    nc.tensor.matmul(
                        yp[:, :],
                        ot[:, t2, ib * 128 : ib * 128 + 128],
                        wfct[:, t2, fcn * 512 : fcn * 512 + 512],
                        start=(t2 == 0),
                        stop=(t2 == 1),
                    )
                ysb = ob.tile([128, 512], F16, tag="ysb")
                # alternate ACT/DVE so neither engine's exp stream starves
                if tail or (ib + fcn) % 2 == 0:
                    nc.scalar.copy(ysb[:, :], yp[:, :])
                else:
                    nc.vector.tensor_copy(ysb[:, :], yp[:, :])
                nc.sync.dma_start(
                    out=y_d[ib * 128 : ib * 128 + 128, fcn * 512 : fcn * 512 + 512],
                    in_=ysb,
                )

            for c in range(16):
                tp = ps.tile([128, DH], F16, tag="mmps", bufs=2)
                nc.tensor.transpose(
                    tp[:, :],
                    kvt[64:128, c * 128 : c * 128 + 128],
                    ident[64:128, 64:128],
                )
                nc.vector.tensor_copy(vo[:, c, 0:DH], tp[:, :])
            nc.vector.memset(vo[:, :, DH : DH + 1], 1.0)

            # PE filler units interleaved with the attention stream: fc(ic-1)
            # lands in block ic (q proj all happened during the input load)
            def _fillers_for(ic):
                units = []
                if ic >= 1:
                    for ib in range(4 * (ic - 1), 4 * ic):
                        for fcn in range(2):
                            units.append(lambda a=ib, b=fcn: _fc_unit(a, b))
                return units

            for ic in range(NIC):
                fill = _fillers_for(ic)
                n_units = len(fill)
                n_ch_tot = 2 * (4 * ic + 4)
                chi = 0
                for t2 in range(2):
                    oa = ps.tile([65, 2, 512], F32, tag="oa", bufs=1)
                    # diagonal chunks first (mask latency hides behind the
                    # off-diagonal work) — except the smallest one (off=384),
                    # which goes last so the pair's closing chain is short
                    order = ([4 * ic + t for t in range(3)]
                             + list(range(4 * ic)) + [4 * ic + 3])
                    n_ch = len(order)

                    def _pv(ent, last):
                        pc, poff, ppt, pidx = ent
                        if use_pv_merge:
                            nc.tensor.matmul(
                                oa[:, :, poff:512],
                                vo[:, pc, 0 : DH + 1],
                                ppt[:, :, poff:512],
                                start=(pidx == 0),
                                stop=last,
                                skip_group_check=True,
                            )
                        else:
                            for h in range(2):
                                nc.tensor.matmul(
                                    oa[:, h, poff:512],
                                    vo[:, pc, 0 : DH + 1],
                                    ppt[:, h, poff:512],
                                    start=(pidx == 0),
                                    stop=(last and h == 1),
                                    skip_group_check=True,
                                )

                    pend = []  # chunk awaiting PV emission (pipeline depth 1)
                    for idx, c in enumerate(order):
                        off = max(0, 128 * c - 512 * ic)
                        stp = ps.tile([128, 2, 512], F32, tag="stp", bufs=2)
                        nc.tensor.matmul(
                            stp[:, 0, off:512],
                            kvt[0:64, c * 128 : c * 128 + 128],
                            qt[0:64, t2, ic * 512 + off : ic * 512 + 512],
                            start=True, stop=True,
                        )
                        nc.tensor.matmul(
                            stp[:, 1, off:512],
                            k2[64:128, c * 128 : c * 128 + 128],
                            qt[64:128, t2, ic * 512 + off : ic * 512 + 512],
                            start=True, stop=True,
                        )
                        pt = wk.tile([128, 2, 512], F16, tag="pt")
                        # exp split: Scalar ACT (exact) on even idx + the
                        # pair-final chunk; custom-DVE cubic on odd idx
                        if use_dve_exp and idx % 2 == 1 and idx != n_ch - 1:
                            nc.vector._custom_dve(
                                exp_op,
                                out=pt[:, :, off:512],
                                in0=stp[:, :, off:512],
                                s0=C3C, s1=C2C, imm2=float(SCALE),
                            )
                        else:
                            nc.scalar.activation(pt[:, :, off:512],
                                                 stp[:, :, off:512], EXP,
                                                 scale=float(SCALE))
                        if c >= 4 * ic:  # diagonal: causal fill on the 128-wide
                            # triangle block (both heads)
                            _pa = pt[:, :, :]
                            _tri = bass.AP(
                                _pa.tensor,
                                _pa.offset + off,
                                [_pa.ap[0], [512, 2], [1, 128]],
                            )
                            nc.gpsimd.affine_select(
                                out=_tri,
                                in_=_tri,
                                compare_op=mybir.AluOpType.is_ge,
                                fill=0.0,
                                base=0,
                                pattern=[[0, 2], [1, 128]],
                                channel_multiplier=-1,
                            )
                        pend.append((c, off, pt, idx))
                        if len(pend) > 1:
                            _pv(pend.pop(0), False)
                        chi += 1
                        # no pops in the pair's last chunk slot (keeps the
                        # closing chain short; deferred units pop next pair)
                        if idx < n_ch - 1:
                            want_left = n_units * (n_ch_tot - chi) // n_ch_tot
                            while len(fill) > want_left:
                                fill.pop(0)()
                    while pend:
                        _pv(pend.pop(0), not pend)
                    # normalize: ot = oa[0:64] / sums (row 64).  The sums
                    # copy stays on the DVE (scalar is strict-FIFO and would
                    # head-block behind queued exps).
                    ssb = wk.tile([1, 2, 512], F32R, tag="ssb")
                    nc.vector.tensor_copy(ssb[:, :, :], oa[64:65, :, :])
                    for h in range(2):
                        bp = ps.tile([128, 512], F32, tag="mmps", bufs=2)
                        nc.tensor.matmul(bp[0:DH, :], ones_row[:, :], ssb[:, h, :],
                                         start=True, stop=True)
                        rinv = wk.tile([DH, 512], F32, tag="rinv")
                        nc.vector.reciprocal_approx_fast(out=rinv[:, :], in_=bp[0:DH, :])
                        nc.vector.tensor_mul(
                            ot[DH * h : DH * h + DH, t2, ic * 512 : ic * 512 + 512],
                            oa[0:DH, h, :],
                            rinv[:, :],
                        )
                for u in fill:
                    u()
            for ib in range(4 * (NIC - 1), 4 * NIC):
                for fcn in range(2):
                    _fc_unit(ib, fcn, tail=True)

    nc.compile()
    return nc


def _numpy_reference(x, mask, Wq, Wk, Wv, Wfc, bfc):
    b, n, _ = x.shape
    q = (x @ Wq.T).reshape(b, n, NH, DH).transpose(0, 2, 1, 3)
    k = x @ Wk.T
    v = x @ Wv.T
    energy = np.einsum("bhid,bjd->bhij", q, k) * SCALE
    mask_value = -np.finfo(energy.dtype).max
    energy = np.where(mask[:, None, :, None], energy, mask_value)
    i = np.arange(n)
    causal = i[:, None] < i[None, :]
    energy = np.where(causal[None, None], mask_value, energy)
    energy = energy - energy.max(axis=-1, keepdims=True)
    attn = np.exp(energy)
    attn = attn / attn.sum(axis=-1, keepdims=True)
    out = np.einsum("bhij,bjd->bhid", attn, v)
    out = out.transpose(0, 2, 1, 3).reshape(b, n, NH * DH)
    return out @ Wfc.T + bfc


def kernel(x, mask, Wq, Wk, Wv, Wfc, bfc):
    global _compiled, _last_results, last_exec_time_ns
    x = np.asarray(x, dtype=np.float32)
    mask = np.asarray(mask)
    Wq = np.asarray(Wq, dtype=np.float32)
    Wk = np.asarray(Wk, dtype=np.float32)
    Wv = np.asarray(Wv, dtype=np.float32)
    Wfc = np.asarray(Wfc, dtype=np.float32)
    bfc = np.asarray(bfc, dtype=np.float32)

    if not mask.all():
        return _numpy_reference(x, mask, Wq, Wk, Wv, Wfc, bfc).astype(np.float32)

    if _compiled is None:
        _compiled = _build()
    nc = _compiled

    wkv_host = np.concatenate([Wk.T, Wv.T], axis=1).astype(np.float16)  # (D, 128)
    wqT = Wq.T.astype(np.float32)  # UNscaled; 1/32 lives in the exp stage
    wfcT = Wfc.T.astype(np.float16)  # (E, D)

    in_maps = []
    for c in range(8):
        b, g = c // 4, c % 4
        e0 = g * HPC * DH
        xTb = np.ascontiguousarray(x[b].T)
        in_maps.append(
            {
                "xT": xTb.astype(np.float16),
                "xT8": xTb.astype(F8NP),
                "wq8": np.ascontiguousarray(wqT[:, e0 : e0 + HPC * DH]).astype(F8NP),
                "wkv": wkv_host,
                "wfc": np.ascontiguousarray(wfcT[e0 : e0 + HPC * DH, :]),
            }
        )

    global _warmed
    if not _warmed:
        # one untraced execute so the measured run sees warm device state
        # (NEFF/TDRAM staging, power state) — steady-state timing
        from concourse import bass2jax
        bass2jax.run_bass_via_pjrt(nc, in_maps, n_cores=8)
        _warmed = True

    trace = bool(int(os.environ.get("KERNEL_TRACE", "0")))
    res = run_bass_kernel_spmd(nc, in_maps, core_ids=list(range(8)), trace=trace)
    _last_results = res
    last_exec_time_ns = res.exec_time_ns

    y = np.empty((NB, N, D), dtype=np.float32)
    for b in range(NB):
        acc = res.results[4 * b]["y"].astype(np.float32)
        for g in range(1, 4):
            acc += res.results[4 * b + g]["y"].astype(np.float32)
        y[b] = acc + bfc
    return y
